# revision 24
# baseline (speedup 1.0000x reference)
"""CrossSpectralDensity kernel for 8 trn2 NeuronCores.

Math: the reference's mean over all FxF pairwise cross-spectra collapses
(by linearity of everything downstream of the einsum) to

    s[b,t]  = sum_f x[b,t,f]
    S[b,k]  = rfft(s, n=8192)[k]
    p[b,k]  = |S[b,k]|^2                       (k = 0..4096)
    out[b,] = (p @ A)                          (A: fixed 4097x256 complex map)

where A = rfft_ortho(irfft(I, n=8192)[:, idx])[:256] / F^2 folds the
irfft -> 'same'-window gather -> ortho rfft -> /F^2 pipeline.

On device (per core, 4 batches, pure data parallel):
  1. DMA x slice, reduce over F=16 on VectorE -> s (128p = 4b x 32a, 128 free)
  2. len-8192 rfft via 64x128 Cooley-Tukey: stage-1 len-64 DFT as matmul
     (only 32 nonzero inputs), twiddle on VectorE, stage-2 len-128 DFT as
     matmuls producing X[c,d] with S[c+64d] = X[c,d], d in [0,65)
  3. p = Xr^2 + Xi^2, repacked to (c + 64*(d%2), (d//2)*4 + b)
  4. out[b,:] = sum over 33 accumulating matmuls of p-chunk.T @ A-chunk
"""

import numpy as np

B, T, FEAT = 32, 4096, 16
NFFT = 2 * T            # 8192
KBINS = NFFT // 2 + 1   # 4097
NF = 256
L = (T - 1) // 2        # 2047
N1, N2 = 64, 128        # t = a*N2 + b_t ;  S[c + 64*d] = X[c, d]
NDD = 33                # d-pair chunks: d = 2*dd + par, d in [0,66), d=65 padded
BPC = 4                 # batches per core
NCORES = 8

_cache = {}


def _constants():
    if "consts" in _cache:
        return _cache["consts"]
    f32 = np.float32
    a = np.arange(32)
    j = np.arange(N1)
    blk_c = np.cos(2 * np.pi * np.outer(a, j) / N1).astype(f32)
    blk_s = (-np.sin(2 * np.pi * np.outer(a, j) / N1)).astype(f32)
    w1 = np.zeros((128, 128), f32)
    for b in range(BPC):
        w1[32 * b:32 * b + 32, :N1] = blk_c
        w1[32 * b:32 * b + 32, N1:] = blk_s

    n = np.arange(N2)[:, None]
    c = np.arange(N1)[None, :]
    th = 2 * np.pi * (n * c) / NFFT
    twr4 = np.tile(np.cos(th).astype(f32), (1, BPC))    # (128, 256)
    twi4 = np.tile((-np.sin(th)).astype(f32), (1, BPC))

    # stage-2 DFT matrices carry a 2^-5 pre-scale so p = X^2 comes out
    # scaled by 2^-10: keeps p in comfortable fp16 range (max ~8e2).
    PSC = 2.0 ** -5
    de = np.arange(NDD)[None, :]
    do = np.arange(32)[None, :]
    th_e = 2 * np.pi * (n * (2 * de)) / N2
    th_o = 2 * np.pi * (n * (2 * do + 1)) / N2
    c2e = (PSC * np.cos(th_e)).astype(f32)
    s2e = (PSC * np.sin(th_e)).astype(f32)
    c2o = (PSC * np.cos(th_o)).astype(f32)
    s2o = (PSC * np.sin(th_o)).astype(f32)

    # A matrix (4097, 512 = [Re|Im]) built numerically from the exact
    # linear pipeline applied to basis vectors (chunked for memory).
    idx = (np.arange(T) - L) % NFFT
    Ag = np.zeros((KBINS, 2 * NF), np.float64)
    CH = 512
    for k0 in range(0, KBINS, CH):
        k1 = min(k0 + CH, KBINS)
        eye = np.zeros((k1 - k0, KBINS))
        eye[np.arange(k1 - k0), np.arange(k0, k1)] = 1.0
        cf = np.fft.irfft(eye, n=NFFT, axis=1)[:, idx]
        Af = np.fft.rfft(cf, axis=1, norm="ortho")[:, :NF] / (FEAT * FEAT)
        Ag[k0:k1, :NF] = Af.real
        Ag[k0:k1, NF:] = Af.imag

    # a2 in fp16, carrying the inverse 2^10 scale (validated: rel err ~3e-4)
    a2 = np.zeros((NDD, 128, 2 * NF), np.float16)
    for dd in range(NDD):
        for par in range(2):
            d = 2 * dd + par
            k = np.arange(N1) + N1 * d
            valid = k <= NFFT // 2
            rows = np.zeros((N1, 2 * NF), np.float64)
            rows[valid] = Ag[k[valid]]
            a2[dd, 64 * par:64 * par + 64, :] = (rows / (PSC * PSC)).astype(np.float16)

    # pack all small fp32 consts into one (128, 835) tensor: one DMA + one
    # DVE copy so every matmul has a single (DVE) dependency source.
    cpack = np.concatenate(
        [w1, twr4, twi4, c2e, s2e, -s2e, c2o, s2o, -s2o], axis=1).astype(f32)
    # a2 flattened to (128, 33*512): column chunk dd*512+j = A2[dd, q, j]
    a2flat = np.ascontiguousarray(np.transpose(a2, (1, 0, 2)).reshape(128, NDD * 2 * NF))
    consts = dict(cpack=cpack, a2=a2flat)
    _cache["consts"] = consts
    return consts


# column offsets inside cpack
_CP = {}
_off = 0
for _name, _w in [("w1", 128), ("twr", 256), ("twi", 256), ("c2e", NDD),
                  ("s2e", NDD), ("s2en", NDD), ("c2o", 32), ("s2o", 32),
                  ("s2on", 32)]:
    _CP[_name] = (_off, _off + _w)
    _off += _w
CPACK_W = _off  # 835


def build_body(tc, io):
    """Emit the per-core kernel. io: dict name -> dram AP."""
    import concourse.mybir as mybir
    nc = tc.nc
    f32 = mybir.dt.float32
    AX = mybir.AxisListType
    OP = mybir.AluOpType

    f16 = mybir.dt.float16
    # Separate pool for DMA-landing tiles: compute tiles must never reuse
    # their (released) space, else the WAR dep lands as a SECOND semaphore
    # wait on a compute instruction, and this walrus build allows only one
    # sync wait per compute instruction.
    with tc.tile_pool(name="dmap", bufs=1) as dmap, \
         tc.tile_pool(name="sb", bufs=1) as sb, \
         tc.tile_pool(name="ps", bufs=1, space="PSUM") as ps:
        # --- constants: one packed DMA + one DVE copy, so every consumer
        # has a single DVE dependency source ---
        cpd = dmap.tile([128, CPACK_W], f32)
        nc.sync.dma_start(out=cpd[:], in_=io["cpack"])
        cp = sb.tile([128, CPACK_W], f32)
        nc.vector.tensor_copy(cp[:], cpd[:])

        def cpv(name):
            a, bnd = _CP[name]
            return cp[:, a:bnd]

        # a2: 4 big DMA chunks + 4 DVE copies (fp16, (128, 33*512))
        a2d = dmap.tile([128, NDD * 2 * NF], f16)
        a2c = sb.tile([128, NDD * 2 * NF], f16)
        bounds = [0, 8 * 512, 16 * 512, 24 * 512, NDD * 512]
        for j in range(4):
            c0, c1 = bounds[j], bounds[j + 1]
            nc.sync.dma_start(out=a2d[:, c0:c1], in_=io["a2"][:, c0:c1])
            nc.vector.tensor_copy(a2c[:, c0:c1], a2d[:, c0:c1])

        # x load (b,a on partitions; (b_t, f) free) + feature reduce
        xt = dmap.tile([128, N2 * FEAT], f32)
        for b in range(BPC):
            nc.sync.dma_start(
                out=xt[32 * b:32 * b + 32, :],
                in_=io["x"][b].rearrange("(a n) f -> a (n f)", a=32))
        s_all = sb.tile([128, N2], f32)
        for b in range(BPC):
            nc.vector.tensor_reduce(
                out=s_all[32 * b:32 * b + 32, :],
                in_=xt[32 * b:32 * b + 32, :].rearrange("p (n f) -> p n f", f=FEAT),
                axis=AX.X, op=OP.add)

        # stage 1: Y^T[b_t, c] per batch (row-group packed)
        ypsr = ps.tile([128, 256], f32)
        ypsi = ps.tile([128, 256], f32)
        w1v = cpv("w1")
        for b in range(BPC):
            lhs = s_all[32 * b:32 * b + 32, :]
            nc.tensor.matmul(ypsr[:, 64 * b:64 * b + 64], lhs,
                             w1v[32 * b:32 * b + 32, 0:N1],
                             start=True, stop=True, tile_position=(32 * b, 0))
            nc.tensor.matmul(ypsi[:, 64 * b:64 * b + 64], lhs,
                             w1v[32 * b:32 * b + 32, N1:128],
                             start=True, stop=True, tile_position=(32 * b, 0))

        # twiddle: Z = Y * tw  -> z (128, [Zr 256 | Zi 256]) SBUF
        # PSUM -> SBUF copies first: each carries the single PE wait, so the
        # multiplies below are all-DVE (waits merge into one DVE semaphore).
        yr_sb = sb.tile([128, 256], f32)
        yi_sb = sb.tile([128, 256], f32)
        nc.vector.tensor_copy(yr_sb[:], ypsr[:])
        nc.vector.tensor_copy(yi_sb[:], ypsi[:])
        z = sb.tile([128, 512], f32)
        tA = sb.tile([128, 256], f32)
        tB = sb.tile([128, 256], f32)
        tC = sb.tile([128, 256], f32)
        tD = sb.tile([128, 256], f32)
        nc.vector.tensor_mul(tA[:], yr_sb[:], cpv("twr"))
        nc.vector.tensor_mul(tB[:], yi_sb[:], cpv("twi"))
        nc.vector.tensor_sub(z[:, 0:256], tA[:], tB[:])
        nc.vector.tensor_mul(tC[:], yr_sb[:], cpv("twi"))
        nc.vector.tensor_mul(tD[:], yi_sb[:], cpv("twr"))
        nc.vector.tensor_add(z[:, 256:512], tC[:], tD[:])

        # stage 2: X[c, d] split by d parity (odd d -> partitions 64..127)
        xrc = ps.tile([128, 132], f32)
        xrs = ps.tile([128, 132], f32)
        xic = ps.tile([128, 132], f32)
        xis = ps.tile([128, 132], f32)
        for b in range(BPC):
            zr = z[:, 64 * b:64 * b + 64]
            zi = z[:, 256 + 64 * b:256 + 64 * b + 64]
            e = slice(NDD * b, NDD * b + NDD)
            o = slice(32 * b, 32 * b + 32)
            nc.tensor.matmul(xrc[0:64, e], zr, cpv("c2e"), start=True, stop=True)
            nc.tensor.matmul(xrs[0:64, e], zi, cpv("s2e"), start=True, stop=True)
            nc.tensor.matmul(xic[0:64, e], zi, cpv("c2e"), start=True, stop=True)
            nc.tensor.matmul(xis[0:64, e], zr, cpv("s2en"), start=True, stop=True)
            nc.tensor.matmul(xrc[64:128, o], zr, cpv("c2o"), start=True, stop=True,
                             tile_position=(0, 64))
            nc.tensor.matmul(xrs[64:128, o], zi, cpv("s2o"), start=True, stop=True,
                             tile_position=(0, 64))
            nc.tensor.matmul(xic[64:128, o], zi, cpv("c2o"), start=True, stop=True,
                             tile_position=(0, 64))
            nc.tensor.matmul(xis[64:128, o], zr, cpv("s2on"), start=True, stop=True,
                             tile_position=(0, 64))

        # p = (xrc+xrs)^2 + (xic+xis)^2, repacked to [q, dd*4 + b]
        # (HW: tensor_tensor may read only ONE input from PSUM -> bounce
        # xrs/xis through SBUF first; on DVE so consumers keep 1 sem source)
        xrs_sb = sb.tile([128, 132], f32)
        xis_sb = sb.tile([128, 132], f32)
        xrc_sb = sb.tile([128, 132], f32)
        xic_sb = sb.tile([128, 132], f32)
        nc.vector.tensor_copy(xrs_sb[0:64, :], xrs[0:64, :])
        nc.vector.tensor_copy(xrs_sb[64:128, 0:128], xrs[64:128, 0:128])
        nc.vector.tensor_copy(xis_sb[0:64, :], xis[0:64, :])
        nc.vector.tensor_copy(xis_sb[64:128, 0:128], xis[64:128, 0:128])
        nc.vector.tensor_copy(xrc_sb[0:64, :], xrc[0:64, :])
        nc.vector.tensor_copy(xrc_sb[64:128, 0:128], xrc[64:128, 0:128])
        nc.vector.tensor_copy(xic_sb[0:64, :], xic[0:64, :])
        nc.vector.tensor_copy(xic_sb[64:128, 0:128], xic[64:128, 0:128])
        t1 = sb.tile([128, 132], f32)
        t2 = sb.tile([128, 132], f32)
        t3 = sb.tile([128, 132], f32)
        t4 = sb.tile([128, 132], f32)
        p2 = sb.tile([128, 132], f16)
        nc.vector.tensor_add(t1[0:64, :], xrc_sb[0:64, :], xrs_sb[0:64, :])
        nc.vector.tensor_add(t1[64:128, 0:128], xrc_sb[64:128, 0:128], xrs_sb[64:128, 0:128])
        nc.vector.tensor_add(t2[0:64, :], xic_sb[0:64, :], xis_sb[0:64, :])
        nc.vector.tensor_add(t2[64:128, 0:128], xic_sb[64:128, 0:128], xis_sb[64:128, 0:128])
        nc.vector.tensor_mul(t3[0:64, :], t1[0:64, :], t1[0:64, :])
        nc.vector.tensor_mul(t3[64:128, 0:128], t1[64:128, 0:128], t1[64:128, 0:128])
        nc.vector.tensor_mul(t4[0:64, :], t2[0:64, :], t2[0:64, :])
        nc.vector.tensor_mul(t4[64:128, 0:128], t2[64:128, 0:128], t2[64:128, 0:128])
        nc.vector.tensor_add(
            p2[0:64, :].rearrange("p (dd b) -> p b dd", b=BPC),
            t3[0:64, :].rearrange("p (b dd) -> p b dd", b=BPC),
            t4[0:64, :].rearrange("p (b dd) -> p b dd", b=BPC))
        nc.vector.tensor_add(
            p2[64:128, 0:128].rearrange("p (dd b) -> p b dd", b=BPC),
            t3[64:128, 0:128].rearrange("p (b dd) -> p b dd", b=BPC),
            t4[64:128, 0:128].rearrange("p (b dd) -> p b dd", b=BPC))
        nc.vector.memset(p2[64:128, 128:132], 0.0)

        # final: out[b, f] = sum_dd p2-chunk.T @ a2-chunk
        ops = ps.tile([BPC, 2 * NF], f32)
        for dd in range(NDD):
            nc.tensor.matmul(ops[:], p2[:, 4 * dd:4 * dd + 4],
                             a2c[:, 512 * dd:512 * dd + 512],
                             start=(dd == 0), stop=(dd == NDD - 1))
        osb = sb.tile([BPC, 2 * NF], f32)
        nc.vector.tensor_copy(osb[:], ops[:])
        nc.gpsimd.dma_start(out=io["out"], in_=osb[:])


def _patch_tail_drain():
    # This walrus build allows only ONE sync wait per instruction; Tile's
    # kernel-tail drain aggregates one wait per semaphore (11 here). Emit
    # single-wait SP nops first — the stock drain's add_sem_waits then
    # elides them as already-observed on the same engine.
    import concourse.tile as tile
    from concourse.vector_clock import ScopedClock
    if getattr(tile.TileContext, "_drain_split_patched", False):
        return
    orig = tile.TileContext._drain_and_barrier

    def patched(self, tick_clock, wait_clock):
        nop0 = self.nc.sync.nop(nofuse=True, hint="drain_wait_split")
        wait_clock.add_sem_waits(
            nop0.ins, ScopedClock({None: tick_clock.global_clock}))
        si = nop0.ins.sync_info
        if si is not None and len(si.on_wait) > 1:
            waits = list(si.on_wait)
            si.on_wait = waits[:1]
            nop0.ins.sync_info = si
            for w in waits[1:]:
                nk = self.nc.sync.nop(nofuse=True, hint="drain_wait_split")
                sik = nk.ins.sync_info
                if sik is None:
                    sik = type(si)(on_wait=[w], on_update=[])
                else:
                    sik.on_wait = [w]
                nk.ins.sync_info = sik
        return orig(self, tick_clock, wait_clock)

    tile.TileContext._drain_and_barrier = patched
    tile.TileContext._drain_split_patched = True


def _build_nc():
    if "nc" in _cache:
        return _cache["nc"]
    import concourse.bass as bass
    import concourse.mybir as mybir
    import concourse.tile as tile
    _patch_tail_drain()
    consts = _constants()
    nc = bass.Bass()
    f32 = mybir.dt.float32
    io = {}
    io["x"] = nc.dram_tensor("x", (BPC, T, FEAT), f32, kind="ExternalInput")[:]
    for name, arr in consts.items():
        dt = mybir.dt.from_np(arr.dtype)
        io[name] = nc.dram_tensor(name, arr.shape, dt, kind="ExternalInput")[:]
    io["out"] = nc.dram_tensor("out", (BPC, 2 * NF), f32, kind="ExternalOutput")[:]
    with tile.TileContext(nc) as tc:
        build_body(tc, io)
    # The tail Drain re-aggregates every outstanding wait; our patch emitted
    # the same wait set as single-wait SP nops immediately before it (same
    # queue, FIFO), so the drain's own waits are redundant — and this walrus
    # build rejects >1 sync wait per instruction.
    for blk in nc.m.functions[0].blocks:
        for ins in blk.instructions:
            si = ins.sync_info
            if si is not None and ins.opcode == "Drain" and len(si.on_wait) > 1:
                si.on_wait = si.on_wait[:1]
                ins.sync_info = si
    _cache["nc"] = nc
    return nc


def _ensure_profile_hook_stub():
    # bass_utils imports antenv.axon_hooks unconditionally when tracing is
    # requested via env; this image's antenv lacks it. A None-hook stub makes
    # that path degrade gracefully instead of crashing.
    try:
        import antenv.axon_hooks  # noqa: F401
    except ImportError:
        import sys
        import types
        m = types.ModuleType("antenv.axon_hooks")
        m.get_axon_ntff_profile_hook = lambda: None
        m.set_axon_ntff_profile_hook = lambda h: None
        sys.modules["antenv.axon_hooks"] = m


def kernel(x):
    _ensure_profile_hook_stub()
    from concourse.bass_utils import run_bass_kernel_spmd
    x = np.ascontiguousarray(np.asarray(x, dtype=np.float32))
    nc = _build_nc()
    consts = _constants()
    in_maps = [dict(x=x[BPC * c:BPC * c + BPC], **consts) for c in range(NCORES)]
    res = run_bass_kernel_spmd(nc, in_maps, list(range(NCORES)))
    _cache["last_result"] = res
    out = np.concatenate([res.results[c]["out"] for c in range(NCORES)], axis=0)
    return (out[:, :NF] + 1j * out[:, NF:]).astype(np.complex64)[:, :, None]
